# revision 27
# baseline (speedup 1.0000x reference)
"""Self-attention (Q=K=V) Trainium2 Bass kernel.

Full input: inputs [8, 2048, 256] fp32.  Output: softmax(X X^T / 16) X,
batched over dim 0.  Sharding: pure data-parallel — one batch element
per NeuronCore (8 cores), no collectives.

Per-core algorithm (X = [2048, 256]):
  - Load X into SBUF row-block tiles (plus two ones columns — fp32r
    needs an even moving width), build X^T on-chip via PE transposes;
    round both to fp32r so every matmul runs at full PE rate.
  - Stage 1 (per 512-wide column group g): for each 128-row block j,
    compute scores with fp32r matmuls and apply exp on the scalar
    engine, storing the unnormalized E = exp(S/16) row-blocks.
  - S is symmetric, so E's row-blocks double as the TRANSPOSED
    probability blocks stage 2 needs as stationary operands — the
    2048x2048 matrix is never transposed.
  - Stage 2 (per 128-query block i of group g): U_i = sum_j E_j[:, i]^T
    @ [X_j | 1].  The appended ones column accumulates the softmax
    denominator in the same PSUM tile, bit-consistent with the
    numerator weights.  Scale by its reciprocal and DMA out.
  - Stage-1 work of group g+1 is interleaved with stage-2 work of
    group g in PE emission order, so the scalar-engine exp stream
    overlaps the PE and the PE never idles (keeps the HAM clock warm).
"""

import numpy as np

import concourse.bacc as bacc
import concourse.tile as tile
from concourse import mybir
from concourse.bass_utils import run_bass_kernel_spmd
from concourse.masks import make_identity

B = 8
N = 2048
D = 256
P = 128
T = N // P   # 16 row/column tiles
C = D // P   # 2 contraction chunks for the scores matmul
G = 4        # 512-wide column groups
GW = N // G  # 512
IPG = T // G  # 4 output tiles per column group
SCALE = 1.0 / 16.0  # 1/sqrt(D)

F32 = mybir.dt.float32
F32R = mybir.dt.float32r
FP8 = mybir.dt.float8e4


def _build_nc():
    nc = bacc.Bacc("TRN2", target_bir_lowering=False, debug=False, num_devices=B)
    x = nc.dram_tensor("x", [N, D], F32, kind="ExternalInput").ap()
    out = nc.dram_tensor("out", [N, D], F32, kind="ExternalOutput").ap()

    with tile.TileContext(nc) as tc:
        with (
            tc.tile_pool(name="big", bufs=1) as big,
            tc.tile_pool(name="small", bufs=1) as small,
            tc.tile_pool(name="psum", bufs=8, space="PSUM") as psum,
            tc.tile_pool(name="ot", bufs=4) as ot,
        ):
            # x_tiles[j][p, 0:256] = X[j*128+p, :]; col 256 = 1.0
            x_tiles = [
                big.tile([P, D + 2], F32, name=f"xj{j}", tag=f"x{j}")
                for j in range(T)
            ]
            xr_tiles = [
                big.tile([P, D + 2], F32R, name=f"xr{j}", tag=f"xr{j}")
                for j in range(T)
            ]
            xt_sb = big.tile([P, C, N], FP8)  # X^T (fp8): xt[p, c, n] = X[n, c*128+p]
            e_sb = big.tile([P, T, N], F32R)   # e_sb[p, j, i] = exp(S[j*128+p, i])

            # Input DMAs first (split across the two HWDGE queues).
            xv = x.rearrange("(t p) d -> p t d", p=P)
            for j in range(T):
                nc.vector.memset(x_tiles[j][:, D : D + 2], 1.0)
            for j in range(T):
                eng = nc.scalar if j % 4 == 1 else nc.sync
                eng.dma_start(out=x_tiles[j][:, 0:D], in_=xv[:, j, :])

            ident = small.tile([P, P], F32)
            make_identity(nc, ident)

            def load_step(j):
                nc.vector.tensor_copy(xr_tiles[j][:], x_tiles[j][:])
                for c in range(C):
                    pt = psum.tile([P, P], F32, tag="ps", name=f"pt{j}_{c}")
                    nc.tensor.transpose(
                        pt[:], x_tiles[j][:, c * P : (c + 1) * P], ident[:]
                    )
                    nc.vector.tensor_copy(xt_sb[:, c, j * P : (j + 1) * P], pt[:])

            def t1_step(g, j):
                """Scores + exp for tile row j, column group g.  fp8
                DoubleRow: one matmul contracts both 128-deep k-subtiles."""
                ps = psum.tile([P, GW], F32, tag="ps", name=f"ps{g}_{j}")
                nc.tensor.matmul(
                    ps[:],
                    lhsT=xt_sb[:, :, j * P : (j + 1) * P],
                    rhs=xt_sb[:, :, g * GW : (g + 1) * GW],
                    start=True,
                    stop=True,
                    perf_mode=mybir.MatmulPerfMode.DoubleRow,
                )
                nc.scalar.activation(
                    out=e_sb[:, j, g * GW : (g + 1) * GW],
                    in_=ps[:],
                    func=mybir.ActivationFunctionType.Exp,
                    scale=SCALE,
                )

            out_r = out.rearrange("(t p) d -> p t d", p=P)
            s2_state = {}

            def s2_mm(g, i, j):
                """One stage-2 accumulation matmul for output tile i."""
                it = g * IPG + i
                if j == 0:
                    s2_state[it] = psum.tile(
                        [P, D + 2], F32, tag="ps", name=f"po{it}"
                    )
                po = s2_state[it]
                nc.tensor.matmul(
                    po[:],
                    lhsT=e_sb[:, j, it * P : (it + 1) * P],
                    rhs=xr_tiles[j][:],
                    start=(j == 0),
                    stop=(j == T - 1),
                )
                if j == T - 1:
                    rl = ot.tile([P, 1], F32, tag="rl", name=f"rl{it}")
                    nc.vector.reciprocal(rl[:], po[:, D : D + 1])
                    o_t = ot.tile([P, D], F32, tag="ot", name=f"o{it}")
                    nc.vector.tensor_scalar_mul(o_t[:], po[:, 0:D], rl[:])
                    nc.sync.dma_start(out=out_r[:, it, :], in_=o_t[:])

            # Software-pipelined emission: T1(g) runs interleaved with S2(g-1).
            # The X^T build is itself interleaved into T1(g0): t1(0, j) only
            # needs X^T blocks 0..3 (its rhs) plus block j (its lhsT).
            for j in range(4):
                load_step(j)
            for j in range(T):
                if j + 4 < T:
                    load_step(j + 4)
                t1_step(0, j)
            for g in range(1, G):
                # 64 S2 matmuls of group g-1 interleaved into 16 T1 steps of g
                s2_list = [(i, j) for i in range(IPG) for j in range(T)]
                for j in range(T):
                    t1_step(g, j)
                    for i2, j2 in s2_list[j * 4 : (j + 1) * 4]:
                        s2_mm(g - 1, i2, j2)
            for i in range(IPG):
                for j in range(T):
                    s2_mm(G - 1, i, j)
            wp = psum.tile([P, P], F32, tag="ps", name="tailwarm")
            nc.tensor.matmul(
                wp[:], lhsT=ident[:], rhs=ident[:], start=True, stop=True
            )

    nc.compile()
    return nc


_NC_CACHE = None
_RUNNER = None


def _make_runner(nc):
    """Build the sharded PJRT callable once (mirrors bass2jax's
    run_bass_via_pjrt) so repeat calls skip jit retracing."""
    import jax
    from jax.sharding import Mesh, PartitionSpec

    from jax.experimental.shard_map import shard_map

    import concourse.bass2jax as b2j
    from concourse import mybir as _mybir

    b2j.install_neuronx_cc_hook()
    partition_name = (
        nc.partition_id_tensor.name if nc.partition_id_tensor else None
    )
    in_names, out_names, out_avals, zero_shapes = [], [], [], []
    for alloc in nc.m.functions[0].allocations:
        if not isinstance(alloc, _mybir.MemoryLocationSet):
            continue
        name = alloc.memorylocations[0].name
        if alloc.kind == "ExternalInput":
            if name != partition_name:
                in_names.append(name)
        elif alloc.kind == "ExternalOutput":
            out_names.append(name)
            shape = tuple(alloc.tensor_shape)
            dtype = _mybir.dt.np(alloc.dtype)
            out_avals.append(jax.core.ShapedArray(shape, dtype))
            zero_shapes.append(((B * shape[0],) + shape[1:], dtype))
    assert in_names == ["x"] and out_names == ["out"]
    n_params = len(in_names)
    all_in_names = list(in_names) + list(out_names)
    if partition_name is not None:
        all_in_names.append(partition_name)
    donate = tuple(range(n_params, n_params + len(out_names)))

    def _body(*args):
        operands = list(args)
        if partition_name is not None:
            operands.append(b2j.partition_id_tensor())
        outs = b2j._bass_exec_p.bind(
            *operands,
            out_avals=tuple(out_avals),
            in_names=tuple(all_in_names),
            out_names=tuple(out_names),
            lowering_input_output_aliases=(),
            sim_require_finite=True,
            sim_require_nnan=True,
            nc=nc,
        )
        return tuple(outs)

    devices = jax.devices()[:B]
    assert len(devices) == B
    mesh = Mesh(np.asarray(devices), ("core",))
    specs = (PartitionSpec("core"),)
    sharded = jax.jit(
        shard_map(
            _body,
            mesh=mesh,
            in_specs=specs * (n_params + len(out_names)),
            out_specs=specs * len(out_names),
            check_rep=False,
        ),
        donate_argnums=donate,
        keep_unused=True,
    )

    def run(x_full: np.ndarray) -> np.ndarray:
        zs = [np.zeros(s, d) for s, d in zero_shapes]
        out = sharded(np.ascontiguousarray(x_full.reshape(B * N, D)), *zs)
        return np.asarray(out[0]).reshape(B, N, D)

    return run


def kernel(inputs: np.ndarray) -> np.ndarray:
    global _NC_CACHE, _RUNNER
    if _NC_CACHE is None:
        _NC_CACHE = _build_nc()
    nc = _NC_CACHE
    inputs = np.ascontiguousarray(np.asarray(inputs, dtype=np.float32))
    assert inputs.shape == (B, N, D)
    if _RUNNER is None:
        try:
            _RUNNER = _make_runner(nc)
        except Exception:
            _RUNNER = False
    if _RUNNER:
        try:
            return _RUNNER(inputs)
        except Exception:
            pass
    in_maps = [{"x": inputs[i]} for i in range(B)]
    res = run_bass_kernel_spmd(nc, in_maps, list(range(B)))
    return np.stack([res.results[i]["out"] for i in range(B)], axis=0)


# revision 30
# speedup vs baseline: 1.1039x; 1.1039x over previous
"""Self-attention (Q=K=V) Trainium2 Bass kernel.

Full input: inputs [8, 2048, 256] fp32.  Output: softmax(X X^T / 16) X,
batched over dim 0.  Sharding: pure data-parallel — one batch element
per NeuronCore (8 cores), no collectives.

Per-core algorithm (X = [2048, 256]):
  - Load X into SBUF row-block tiles (plus two ones columns — fp32r
    needs an even moving width), build X^T on-chip via PE transposes;
    round both to fp32r so every matmul runs at full PE rate.
  - Stage 1 (per 512-wide column group g): for each 128-row block j,
    compute scores with fp32r matmuls and apply exp on the scalar
    engine, storing the unnormalized E = exp(S/16) row-blocks.
  - S is symmetric, so E's row-blocks double as the TRANSPOSED
    probability blocks stage 2 needs as stationary operands — the
    2048x2048 matrix is never transposed.
  - Stage 2 (per 128-query block i of group g): U_i = sum_j E_j[:, i]^T
    @ [X_j | 1].  The appended ones column accumulates the softmax
    denominator in the same PSUM tile, bit-consistent with the
    numerator weights.  Scale by its reciprocal and DMA out.
  - Stage-1 work of group g+1 is interleaved with stage-2 work of
    group g in PE emission order, so the scalar-engine exp stream
    overlaps the PE and the PE never idles (keeps the HAM clock warm).
"""

import numpy as np

import concourse.bacc as bacc
import concourse.tile as tile
from concourse import mybir
from concourse.bass_utils import run_bass_kernel_spmd
from concourse.masks import make_identity

B = 8
N = 2048
D = 256
P = 128
T = N // P   # 16 row/column tiles
C = D // P   # 2 contraction chunks for the scores matmul
G = 4        # 512-wide column groups
GW = N // G  # 512
IPG = T // G  # 4 output tiles per column group
SCALE = 1.0 / 16.0  # 1/sqrt(D)
EBIAS = -2.772588722239781  # -ln(16): store E/16 so off-diag fits fp8e4

F32 = mybir.dt.float32
F32R = mybir.dt.float32r
FP8 = mybir.dt.float8e4


def _build_nc():
    nc = bacc.Bacc("TRN2", target_bir_lowering=False, debug=False, num_devices=B)
    x = nc.dram_tensor("x", [N, D], F32, kind="ExternalInput").ap()
    out = nc.dram_tensor("out", [N, D], F32, kind="ExternalOutput").ap()

    with tile.TileContext(nc) as tc:
        with (
            tc.tile_pool(name="big", bufs=1) as big,
            tc.tile_pool(name="small", bufs=1) as small,
            tc.tile_pool(name="psum", bufs=8, space="PSUM") as psum,
            tc.tile_pool(name="ot", bufs=4) as ot,
        ):
            # x_tiles[j][p, 0:256] = X[j*128+p, :]; col 256 = 1.0
            x_tiles = [
                big.tile([P, D + 2], F32, name=f"xj{j}", tag=f"x{j}")
                for j in range(T)
            ]
            xr_tiles = [
                big.tile([P, D + 2], F32R, name=f"xr{j}", tag=f"xr{j}")
                for j in range(T)
            ]
            xt_sb = big.tile([P, C, N], FP8)  # X^T (fp8): xt[p, c, n] = X[n, c*128+p]
            # E/16 storage, split by consumer precision: the tile of row j
            # that contains the diagonal block (columns of group j//4) stays
            # f32r; all off-diagonal tiles are fp8 (their weights carry only
            # ~4e-4 of the softmax mass).
            e32 = big.tile([P, T, GW], F32R)  # e32[p, j, :] = cols of group j//4
            e8 = big.tile([P, T, N], FP8)
            # fp8 x pairs for DoubleRow stage-2: x8p[jp][p,h,:] = row block 2jp+h
            x8p = [
                big.tile([P, 2, D + 2], FP8, name=f"x8p{jp}", tag=f"x8p{jp}")
                for jp in range(T // 2)
            ]

            # Input DMAs first (split across the two HWDGE queues).
            xv = x.rearrange("(t p) d -> p t d", p=P)
            for j in range(T):
                nc.vector.memset(x_tiles[j][:, D : D + 2], 1.0)
            for j in range(T):
                eng = nc.sync if j % 2 == 0 else nc.scalar
                eng.dma_start(out=x_tiles[j][:, 0:D], in_=xv[:, j, :])

            ident = small.tile([P, P], F32)
            make_identity(nc, ident)
            ebias = small.tile([P, 1], F32)
            nc.vector.memset(ebias[:], EBIAS)

            def load_step(j):
                nc.vector.tensor_copy(xr_tiles[j][:], x_tiles[j][:])
                nc.vector.tensor_copy(x8p[j // 2][:, j % 2, :], x_tiles[j][:])
                for c in range(C):
                    pt = psum.tile([P, P], F32, tag="ps", name=f"pt{j}_{c}")
                    nc.tensor.transpose(
                        pt[:], x_tiles[j][:, c * P : (c + 1) * P], ident[:]
                    )
                    nc.vector.tensor_copy(xt_sb[:, c, j * P : (j + 1) * P], pt[:])

            def t1_step(g, j):
                """Scores + exp for tile row j, column group g.  fp8
                DoubleRow: one matmul contracts both 128-deep k-subtiles."""
                ps = psum.tile([P, GW], F32, tag="ps", name=f"ps{g}_{j}")
                nc.tensor.matmul(
                    ps[:],
                    lhsT=xt_sb[:, :, j * P : (j + 1) * P],
                    rhs=xt_sb[:, :, g * GW : (g + 1) * GW],
                    start=True,
                    stop=True,
                    perf_mode=mybir.MatmulPerfMode.DoubleRow,
                )
                dst = (
                    e32[:, j, :]
                    if g == j // IPG
                    else e8[:, j, g * GW : (g + 1) * GW]
                )
                nc.scalar.activation(
                    out=dst,
                    in_=ps[:],
                    func=mybir.ActivationFunctionType.Exp,
                    scale=SCALE,
                    bias=ebias[:],
                )

            out_r = out.rearrange("(t p) d -> p t d", p=P)
            s2_state = {}

            def s2_mms_for(it):
                """Emission list for output tile it: 6 fp8 DoubleRow pair
                matmuls (off-diagonal groups) + 4 f32r matmuls (the group
                containing the diagonal), then normalize + DMA out."""
                g = it // IPG
                steps = []
                for jp in range(T // 2):
                    if jp // 2 != g:  # pair (2jp, 2jp+1) outside diag group
                        steps.append(("fp8", jp))
                for j in range(g * IPG, (g + 1) * IPG):
                    steps.append(("f32r", j))
                return steps

            def s2_mm(it, k):
                steps = s2_mms_for(it)
                kind, v = steps[k]
                if k == 0:
                    s2_state[it] = psum.tile(
                        [P, D + 2], F32, tag="ps", name=f"po{it}"
                    )
                po = s2_state[it]
                if kind == "fp8":
                    nc.tensor.matmul(
                        po[:],
                        lhsT=e8[:, 2 * v : 2 * v + 2, it * P : (it + 1) * P],
                        rhs=x8p[v][:],
                        start=(k == 0),
                        stop=(k == len(steps) - 1),
                        perf_mode=mybir.MatmulPerfMode.DoubleRow,
                    )
                else:
                    lo = (it % IPG) * P
                    nc.tensor.matmul(
                        po[:],
                        lhsT=e32[:, v, lo : lo + P],
                        rhs=xr_tiles[v][:],
                        start=(k == 0),
                        stop=(k == len(steps) - 1),
                    )
                if k == len(steps) - 1:
                    rl = ot.tile([P, 1], F32, tag="rl", name=f"rl{it}")
                    nc.vector.reciprocal(rl[:], po[:, D : D + 1])
                    o_t = ot.tile([P, D], F32, tag="ot", name=f"o{it}")
                    nc.vector.tensor_scalar_mul(o_t[:], po[:, 0:D], rl[:])
                    nc.sync.dma_start(out=out_r[:, it, :], in_=o_t[:])

            # Software-pipelined emission: T1(g) runs interleaved with S2(g-1).
            # The X^T build is itself interleaved into T1(g0): t1(0, j) only
            # needs X^T blocks 0..3 (its rhs) plus block j (its lhsT).
            for j in range(4):
                load_step(j)
            for j in range(T):
                if j + 4 < T:
                    load_step(j + 4)
                t1_step(0, j)
            NS2 = 10  # stage-2 matmuls per output tile (6 fp8 + 4 f32r)
            for g in range(1, G):
                # 40 S2 matmuls of group g-1 interleaved into 16 T1 steps of g
                s2_list = [
                    ((g - 1) * IPG + i, k) for i in range(IPG) for k in range(NS2)
                ]
                for j in range(T):
                    lo = (j * len(s2_list)) // T
                    hi = ((j + 1) * len(s2_list)) // T
                    t1_step(g, j)
                    for it2, k2 in s2_list[lo:hi]:
                        s2_mm(it2, k2)
            for i in range(IPG):
                for k in range(NS2):
                    s2_mm((G - 1) * IPG + i, k)
            wp = psum.tile([P, P], F32, tag="ps", name="tailwarm")
            nc.tensor.matmul(
                wp[:], lhsT=ident[:], rhs=ident[:], start=True, stop=True
            )

    nc.compile()
    return nc


_NC_CACHE = None
_RUNNER = None


def _make_runner(nc):
    """Build the sharded PJRT callable once (mirrors bass2jax's
    run_bass_via_pjrt) so repeat calls skip jit retracing."""
    import jax
    from jax.sharding import Mesh, PartitionSpec

    from jax.experimental.shard_map import shard_map

    import concourse.bass2jax as b2j
    from concourse import mybir as _mybir

    b2j.install_neuronx_cc_hook()
    partition_name = (
        nc.partition_id_tensor.name if nc.partition_id_tensor else None
    )
    in_names, out_names, out_avals, zero_shapes = [], [], [], []
    for alloc in nc.m.functions[0].allocations:
        if not isinstance(alloc, _mybir.MemoryLocationSet):
            continue
        name = alloc.memorylocations[0].name
        if alloc.kind == "ExternalInput":
            if name != partition_name:
                in_names.append(name)
        elif alloc.kind == "ExternalOutput":
            out_names.append(name)
            shape = tuple(alloc.tensor_shape)
            dtype = _mybir.dt.np(alloc.dtype)
            out_avals.append(jax.core.ShapedArray(shape, dtype))
            zero_shapes.append(((B * shape[0],) + shape[1:], dtype))
    assert in_names == ["x"] and out_names == ["out"]
    n_params = len(in_names)
    all_in_names = list(in_names) + list(out_names)
    if partition_name is not None:
        all_in_names.append(partition_name)
    donate = tuple(range(n_params, n_params + len(out_names)))

    def _body(*args):
        operands = list(args)
        if partition_name is not None:
            operands.append(b2j.partition_id_tensor())
        outs = b2j._bass_exec_p.bind(
            *operands,
            out_avals=tuple(out_avals),
            in_names=tuple(all_in_names),
            out_names=tuple(out_names),
            lowering_input_output_aliases=(),
            sim_require_finite=True,
            sim_require_nnan=True,
            nc=nc,
        )
        return tuple(outs)

    devices = jax.devices()[:B]
    assert len(devices) == B
    mesh = Mesh(np.asarray(devices), ("core",))
    specs = (PartitionSpec("core"),)
    sharded = jax.jit(
        shard_map(
            _body,
            mesh=mesh,
            in_specs=specs * (n_params + len(out_names)),
            out_specs=specs * len(out_names),
            check_rep=False,
        ),
        donate_argnums=donate,
        keep_unused=True,
    )

    def run(x_full: np.ndarray) -> np.ndarray:
        zs = [np.zeros(s, d) for s, d in zero_shapes]
        out = sharded(np.ascontiguousarray(x_full.reshape(B * N, D)), *zs)
        return np.asarray(out[0]).reshape(B, N, D)

    return run


def kernel(inputs: np.ndarray) -> np.ndarray:
    global _NC_CACHE, _RUNNER
    if _NC_CACHE is None:
        _NC_CACHE = _build_nc()
    nc = _NC_CACHE
    inputs = np.ascontiguousarray(np.asarray(inputs, dtype=np.float32))
    assert inputs.shape == (B, N, D)
    if _RUNNER is None:
        try:
            _RUNNER = _make_runner(nc)
        except Exception:
            _RUNNER = False
    if _RUNNER:
        try:
            return _RUNNER(inputs)
        except Exception:
            pass
    in_maps = [{"x": inputs[i]} for i in range(B)]
    res = run_bass_kernel_spmd(nc, in_maps, list(range(B)))
    return np.stack([res.results[i]["out"] for i in range(B)], axis=0)
